# revision 1
# baseline (speedup 1.0000x reference)
"""Trainium2 Bass kernel for nn_PrototypicalGeometricLoss.

Strategy (8 NeuronCores, single NEFF launch):
  - Data-parallel streaming: each core mean-pools + L2-normalizes its B/8 = 512
    batch rows of geometric_stream (the memory-bound 134 MB read).  Pooling is
    a 6-step contiguous add-tree on DVE (hidden under the DMA stream).
  - Pooled/normalized embeddings are transposed on-chip ([D, b] tiles) and
    AllGathered (2 MB total) so every core holds gT = [128, 4096].
  - Prototype EMA update is class-sharded (C/8 = 1250 classes per core).
    Segment sums use a gather/scan/gather trick: ap_gather the class-sorted
    member columns of gT, prefix-scan along the free axis (DVE
    tensor_tensor_scan), ap_gather the per-class cumulative endpoints, and a
    shifted subtract yields all class sums.  EMA + renorm runs per 128-class
    tile ([C,D] layout via PE transpose) and is transposed back into
    pT = [D, 1280].
  - Distance phase is class-sharded: every core computes d(b, c) for all 4096
    b x its 1250 classes via PE matmuls (float32r) + one ACT Sqrt pass per
    b-tile (d = sqrt(2 - 2*g.p); g is pre-scaled by (1-1e-6) so the sqrt
    argument stays strictly positive), accumulating sum(d) per partition.
  - Own-class dot products reuse the member gather for the g side, gather the
    p side, and reduce via PE matmul diagonals; raw dots return to the host
    which sqrts them in float64 for exact intra_loss.
  - Host combines per-core partial sums (float64) into the six loss scalars.
"""

import functools
import sys

sys.path.insert(0, "/opt/trn_rl_repo")

import numpy as np

import concourse.bass as bass  # noqa: F401
import concourse.bacc as bacc
import concourse.mybir as mybir
from concourse import tile
from concourse.bass_utils import run_bass_kernel_spmd

N_CORES = 8
B, S, D, C = 4096, 64, 128, 10000
BSH = B // N_CORES           # 512 batch rows per core
LTB = BSH // 128             # 4 local b-tiles
NTB = B // 128               # 32 global b-tiles
CSH = C // N_CORES           # 1250 classes per core
CPAD = 1280                  # padded to 10 x 128
CK = CPAD // 128             # 10 class tiles
GCOL = NTB * 128             # 4096 columns of gT (last col is zero)
EGL = CPAD + 16              # endpoint-gather length (c=-1 slot + 15 pads)
MOM = 0.9
GSCALE = 1.0 - 1e-6          # keeps 2 - 2*g.p strictly positive
EPS = 1e-12

f32 = mybir.dt.float32
f32r = mybir.dt.float32r
i16 = mybir.dt.int16
AF = mybir.ActivationFunctionType
ALU = mybir.AluOpType
AX = mybir.AxisListType


def _wrap16(flat):
    """Lay a flat index list out in the GPSIMD wrapped-by-16 format."""
    n = flat.shape[0]
    assert n % 16 == 0
    w = flat.reshape(n // 16, 16).T.astype(np.int16)   # [16, n//16]
    return np.tile(w, (8, 1))                          # [128, n//16]


@functools.lru_cache(maxsize=16)
def _build(NOCH, use_f32r=True, upto=99, unroll=1, skip_ag=False):
    """Build + compile the SPMD program. NOCH = own-dot chunks of 128."""
    NOWN = NOCH * 128
    MGL = NOWN + 16          # member-gather length (leading zero + pads)
    mm_dt = f32r if use_f32r else f32
    nc = bacc.Bacc("TRN2", target_bir_lowering=False, debug=False,
                   num_devices=N_CORES)

    gs = nc.dram_tensor("gs", [BSH, S * D], f32, kind="ExternalInput")
    pr = nc.dram_tensor("pr", [CPAD, D], f32, kind="ExternalInput")
    sv = nc.dram_tensor("sv", [128, BSH // 128], f32, kind="ExternalInput")
    ss = nc.dram_tensor("ss", [128, CK], f32, kind="ExternalInput")
    mgi = nc.dram_tensor("mgi", [128, MGL // 16], i16, kind="ExternalInput")
    e2i = nc.dram_tensor("e2i", [128, EGL // 16], i16, kind="ExternalInput")
    opi = nc.dram_tensor("opi", [128, NOWN // 16], i16, kind="ExternalInput")
    idn = nc.dram_tensor("idn", [128, 128], f32, kind="ExternalInput")
    po = nc.dram_tensor("po", [128, 4], f32, kind="ExternalOutput")
    oo = nc.dram_tensor("oo", [128, NOCH], f32, kind="ExternalOutput")

    # Uniform labels give NOCH ~5-6; heavily skewed distributions grow the
    # member/own buffers, so shrink streaming buffers to stay within SBUF.
    slab_bufs = 3 if NOCH <= 10 else 2
    norm_bufs = 2 if NOCH <= 10 else 1
    with tile.TileContext(nc) as tc:
        with (
            tc.tile_pool(name="dram", bufs=1, space="DRAM") as dram,
            tc.tile_pool(name="consts", bufs=1) as consts,
            tc.tile_pool(name="gbig", bufs=1) as gbig,
            tc.tile_pool(name="slab", bufs=slab_bufs) as slabp,
            tc.tile_pool(name="norm", bufs=norm_bufs) as normp,
            tc.tile_pool(name="ps_small", bufs=2, space="PSUM") as ps_small,
            tc.tile_pool(name="ps_dist", bufs=2, space="PSUM") as ps_dist,
            tc.tile_pool(name="dscr", bufs=2) as dscrp,
            tc.tile_pool(name="outs", bufs=1) as outsp,
        ):
            for it in range(unroll):
                if it > 0:
                    tc.strict_bb_all_engine_barrier()
                ag_in = dram.tile([LTB, 128, 128], f32, name="ag_in",
                                  tag="ag_in")
                ag_out = dram.tile([NTB, 128, 128], f32, name="ag_out",
                                   addr_space="Shared", tag="ag_out")
                ident = consts.tile([128, 128], f32, name="ident")
                nc.sync.dma_start(ident[:, :], idn[:, :])
                ss_sb = consts.tile([128, CK], f32, name="ss_sb")
                nc.sync.dma_start(ss_sb[:, :], ss[:, :])
                mgi_sb = consts.tile([128, MGL // 16], i16, name="mgi_sb")
                nc.sync.dma_start(mgi_sb[:, :], mgi[:, :])
                e2i_sb = consts.tile([128, EGL // 16], i16, name="e2i_sb")
                nc.sync.dma_start(e2i_sb[:, :], e2i[:, :])
                opi_sb = consts.tile([128, NOWN // 16], i16, name="opi_sb")
                nc.sync.dma_start(opi_sb[:, :], opi[:, :])
                sv_sb = consts.tile([128, BSH // 128], f32, name="sv_sb")
                nc.sync.dma_start(sv_sb[:, :], sv[:, :])
                bias2 = consts.tile([128, 1], f32, name="bias2")
                nc.vector.memset(bias2[:, :], 2.0)
                pr09 = consts.tile([128, CK * 128], f32, name="pr09")
                nc.sync.dma_start(pr09[:, :].rearrange("p (k d) -> p k d", k=CK),
                                  pr[:, :].rearrange("(k p) d -> p k d", k=CK))
                nc.vector.tensor_scalar_mul(pr09[:, :], pr09[:, :], MOM)

                gT = gbig.tile([128, GCOL + 1], mm_dt, name="gT")
                gTf = gT[:, :].bitcast(f32)
                nc.vector.memset(gTf[:, GCOL:GCOL + 1], 0.0)

                # ---- Phase A: stream + pool + normalize + transpose
                for t in range(LTB if upto >= 1 else 0):
                    slab = slabp.tile([128, S * D], f32, name="slab", tag="slab")
                    nc.sync.dma_start(slab[:, :], gs[t * 128:(t + 1) * 128, :])
                    pooled = normp.tile([128, 128], f32, name="pooled", tag="pooled")
                    half = normp.tile([128, S * D // 2], f32, name="half", tag="half")
                    nc.vector.tensor_add(half[:, 0:2048], slab[:, 0:2048], slab[:, 4096:6144])
                    nc.gpsimd.tensor_tensor(half[:, 2048:4096], slab[:, 2048:4096], slab[:, 6144:8192], ALU.add)
                    nc.vector.tensor_add(half[:, 0:1024], half[:, 0:1024], half[:, 2048:3072])
                    nc.gpsimd.tensor_tensor(half[:, 1024:2048], half[:, 1024:2048], half[:, 3072:4096], ALU.add)
                    nc.vector.tensor_add(half[:, 0:1024], half[:, 0:1024], half[:, 1024:2048])
                    nc.vector.tensor_add(half[:, 0:512], half[:, 0:512], half[:, 512:1024])
                    nc.vector.tensor_add(half[:, 0:256], half[:, 0:256], half[:, 256:512])
                    nc.vector.tensor_add(pooled[:, :], half[:, 0:128], half[:, 128:256])
                    scr = normp.tile([128, 128], f32, name="scr", tag="scr")
                    ssq = normp.tile([128, 1], f32, name="ssq", tag="ssq")
                    nc.scalar.activation(scr[:, :], pooled[:, :], AF.Square,
                                         accum_out=ssq[:, :])
                    nrm = normp.tile([128, 1], f32, name="nrm", tag="nrm")
                    nc.scalar.activation(nrm[:, :], ssq[:, :], AF.Sqrt)
                    nc.vector.tensor_scalar_max(nrm[:, :], nrm[:, :], EPS)
                    rcp = normp.tile([128, 1], f32, name="rcp", tag="rcp")
                    nc.vector.reciprocal(rcp[:, :], nrm[:, :])
                    gn = normp.tile([128, 128], f32, name="gn", tag="gn")
                    nc.vector.tensor_scalar(gn[:, :], pooled[:, :], rcp[:, :],
                                            GSCALE, ALU.mult, ALU.mult)
                    pst = ps_small.tile([128, 128], f32, name="pst", tag="pst")
                    nc.tensor.transpose(pst[:, :], gn[:, :], ident[:, :])
                    gloc = normp.tile([128, 128], f32, name="gloc", tag="gloc")
                    nc.scalar.activation(gloc[:, :], pst[:, :], AF.Copy)
                    nc.sync.dma_start(ag_in[t, :, :], gloc[:, :])

                # ---- simplex volume partials
                out_sb = outsp.tile([128, 4], f32, name="out_sb")
                nc.vector.tensor_reduce(out_sb[:, 1:2], sv_sb[:, :], AX.X, ALU.add)
                junk1 = outsp.tile([128, BSH // 128], f32, name="junk1")
                nc.vector.scalar_tensor_tensor(junk1[:, :], sv_sb[:, :], 1.0,
                                               sv_sb[:, :], ALU.mult, ALU.mult,
                                               accum_out=out_sb[:, 2:3])
                nc.vector.memset(out_sb[:, 3:4], 0.0)

                # ---- Phase B: AllGather g tiles; load full gT
                if upto < 1:
                    for t in range(LTB):
                        nc.sync.dma_start(ag_in[t, :, :], ident[:, :])
                def load_gt():
                    for ch in range(8):
                        nc.sync.dma_start(
                            gTf[:, ch * 512:(ch + 1) * 512].rearrange(
                                "p (t c) -> p t c", t=4),
                            ag_out[ch * 4:(ch + 1) * 4, :, :].rearrange(
                                "t p c -> p t c"))
                if upto >= 2 and skip_ag:
                    load_gt()
                if upto >= 2 and not skip_ag:
                    nc.gpsimd.collective_compute(
                        "AllGather", ALU.bypass,
                        replica_groups=[list(range(N_CORES))],
                        ins=[ag_in.opt()], outs=[ag_out.opt()])
                    load_gt()
                if upto < 2:
                    nc.vector.memset(gTf[:, 0:GCOL], 0.0)

                # ---- Phase C: segment sums via gather + prefix scan + gather
                mems = gbig.tile([128, MGL], f32, name="mems")
                cum = gbig.tile([128, MGL], f32, name="cum")
                fx = gbig.tile([128, EGL], f32, name="fx")
                sums = gbig.tile([128, CPAD], f32, name="sums")
                if upto >= 3:
                    nc.gpsimd.ap_gather(mems[:, :], gTf[:, :], mgi_sb[:, :],
                                        channels=128, num_elems=GCOL + 1, d=1,
                                        num_idxs=MGL)
                    nc.vector.tensor_tensor_scan(cum[:, :], mems[:, :], mems[:, :],
                                                 0.0, ALU.add, ALU.bypass)
                    # two overlapping endpoint gathers so phase D can start
                    # on the first five class tiles while the second runs
                    nc.gpsimd.ap_gather(fx[:, 0:656], cum[:, :], e2i_sb[:, 0:41],
                                        channels=128, num_elems=MGL, d=1,
                                        num_idxs=656)
                    nc.vector.tensor_sub(sums[:, 0:640], fx[:, 1:641], fx[:, 0:640])
                    nc.gpsimd.ap_gather(fx[:, 640:1296], cum[:, :], e2i_sb[:, 40:81],
                                        channels=128, num_elems=MGL, d=1,
                                        num_idxs=656)
                    nc.vector.tensor_sub(sums[:, 640:CPAD], fx[:, 641:CPAD + 1],
                                         fx[:, 640:CPAD])
                else:
                    nc.vector.memset(mems[:, :], 0.0)
                    nc.vector.memset(sums[:, :], 0.0)

                # ---- Phase D: prototype EMA + renorm, produce pT [D, CPAD]
                ptil = gbig.tile([128, CPAD], f32, name="ptil")
                ssqall = outsp.tile([128, CK], f32, name="ssqall")
                dscr0 = outsp.tile([128, 128], f32, name="dscr0")
                if upto < 4:
                    nc.vector.memset(ptil[:, :], 0.0)
                    nc.vector.memset(ssqall[:, :], 1.0)
                for k in range(CK if upto >= 4 else 0):
                    pst2 = ps_small.tile([128, 128], f32, name="pst2", tag="pst")
                    nc.tensor.transpose(pst2[:, :], sums[:, k * 128:(k + 1) * 128],
                                        ident[:, :])
                    nc.vector.scalar_tensor_tensor(
                        ptil[:, k * 128:(k + 1) * 128], pst2[:, :],
                        ss_sb[:, k:k + 1],
                        pr09[:, k * 128:(k + 1) * 128],
                        ALU.mult, ALU.add)
                    nc.scalar.activation(dscr0[:, :], ptil[:, k * 128:(k + 1) * 128],
                                         AF.Square, accum_out=ssqall[:, k:k + 1])
                nrmall = outsp.tile([128, CK], f32, name="nrmall")
                nc.scalar.activation(nrmall[:, :], ssqall[:, :], AF.Sqrt)
                nc.vector.tensor_scalar_max(nrmall[:, :], nrmall[:, :], EPS)
                rcpall = outsp.tile([128, CK], f32, name="rcpall")
                nc.vector.reciprocal(rcpall[:, :], nrmall[:, :])
                pT = gbig.tile([128, CPAD], mm_dt, name="pT")
                pTf = pT[:, :].bitcast(f32)
                if upto < 4:
                    nc.vector.memset(pTf[:, :], 0.0)
                for k in range(CK if upto >= 4 else 0):
                    nc.vector.tensor_scalar_mul(ptil[:, k * 128:(k + 1) * 128],
                                                ptil[:, k * 128:(k + 1) * 128],
                                                rcpall[:, k:k + 1])
                    pst3 = ps_small.tile([128, 128], f32, name="pst3", tag="pst")
                    nc.tensor.transpose(pst3[:, :], ptil[:, k * 128:(k + 1) * 128],
                                        ident[:, :])
                    nc.scalar.activation(pT[:, k * 128:(k + 1) * 128], pst3[:, :],
                                         AF.Copy)

                # ---- Phase E: own-class raw dot products (g side = mems)
                opg = gbig.tile([128, NOWN], f32, name="opg")
                if upto >= 5:
                    nc.gpsimd.ap_gather(opg[:, :], pTf[:, :], opi_sb[:, :],
                                        channels=128, num_elems=CPAD, d=1,
                                        num_idxs=NOWN)
                dots = outsp.tile([128, NOCH], f32, name="dots")
                junk2 = outsp.tile([128, 128], f32, name="junk2")
                if upto < 5:
                    nc.vector.memset(dots[:, :], 0.0)
                    nc.vector.memset(opg[:, :], 0.0)
                for cc in range(NOCH if upto >= 5 else 0):
                    psd = ps_small.tile([128, 128], f32, name="psd", tag="pst")
                    nc.tensor.matmul(psd[:, :],
                                     mems[:, 1 + cc * 128:1 + (cc + 1) * 128],
                                     opg[:, cc * 128:(cc + 1) * 128],
                                     start=True, stop=True)
                    nc.vector.scalar_tensor_tensor(
                        junk2[:, :], psd[:, :], 1.0, ident[:, :],
                        ALU.mult, ALU.mult, accum_out=dots[:, cc:cc + 1])
                nc.sync.dma_start(oo[:, :], dots[:, :])

                # ---- Phase F: distance matmuls + sqrt-accumulate
                acc = outsp.tile([128, NTB], f32, name="acc")
                if upto < 6:
                    nc.vector.memset(acc[:, :], 0.0)
                for bt in range(NTB if upto >= 6 else 0):
                    psf = ps_dist.tile([128, CPAD], f32, name="psf", tag="psf")
                    lhs = gT[:, bt * 128:(bt + 1) * 128]
                    for c0, cn in ((0, 512), (512, 512), (1024, 256)):
                        nc.tensor.matmul(psf[:, c0:c0 + cn], lhs, pT[:, c0:c0 + cn],
                                         start=True, stop=True)
                    dsc = dscrp.tile([128, CSH], f32, name="dsc", tag="dsc")
                    nc.scalar.activation(dsc[:, :], psf[:, 0:CSH], AF.Sqrt,
                                         bias=bias2[:, :], scale=-2.0)
                    nc.vector.tensor_reduce(acc[:, bt:bt + 1], dsc[:, :],
                                            AX.X, ALU.add)
                nc.vector.tensor_reduce(out_sb[:, 0:1], acc[:, :], AX.X, ALU.add)
                nc.sync.dma_start(po[:, :], out_sb[:, :])

    nc.compile()
    return nc


def _prep(geometric_stream, simplex_volumes, prototypes, labels):
    gs = np.ascontiguousarray(np.asarray(geometric_stream, dtype=np.float32))
    svol = np.ascontiguousarray(np.asarray(simplex_volumes, dtype=np.float32))
    pr = np.asarray(prototypes, dtype=np.float32)
    lab = np.asarray(labels).astype(np.int64).ravel()
    assert gs.shape == (B, S, D) and pr.shape == (C, D) and lab.shape == (B,)

    counts = np.bincount(lab, minlength=C)
    sscale = ((1.0 - MOM) / np.maximum(counts, 1.0)).astype(np.float32)

    shard_of = lab // CSH
    n_own = np.bincount(shard_of, minlength=N_CORES)
    NOCH = max(1, int(-(-n_own.max() // 128)))
    NOWN = NOCH * 128
    MGL = NOWN + 16

    in_maps = []
    own_b = []   # per core: batch indices in (class, b) order
    ident = np.eye(128, dtype=np.float32)
    for j in range(N_CORES):
        c0 = j * CSH
        sel = shard_of == j
        bsel = np.nonzero(sel)[0]
        slots = lab[bsel] - c0
        srt = np.lexsort((bsel, slots))
        bsel, slots = bsel[srt], slots[srt]
        n_j = len(bsel)

        # member gather: [zero] + sorted member columns + zero pads
        mg = np.full(MGL, GCOL, dtype=np.int64)
        mg[1:1 + n_j] = bsel
        # endpoint gather: position of cumulative sum after each class
        # e2[0] = 0 (cum col 0 == 0); e2[c+1] = 1 + #members with slot <= c
        m_c = np.cumsum(np.bincount(slots, minlength=CPAD))
        e2 = np.zeros(EGL, dtype=np.int64)
        e2[1:CPAD + 1] = m_c   # cum col m_c is the inclusive sum of members 1..m_c
        e2[CPAD + 1:] = m_c[-1]
        # own-p gather: dense class slot per member (zero col of pT for pads)
        opf = np.full(NOWN, CPAD - 1, dtype=np.int64)
        opf[:n_j] = slots

        prj = np.zeros((CPAD, D), dtype=np.float32)
        prj[:CSH] = pr[c0:c0 + CSH]
        ssj = np.zeros(CPAD, dtype=np.float32)
        ssj[:CSH] = sscale[c0:c0 + CSH]

        in_maps.append({
            "gs": gs[BSH * j:BSH * (j + 1)].reshape(BSH, S * D),
            "pr": prj,
            "sv": svol[BSH * j:BSH * (j + 1)].reshape(128, BSH // 128),
            "ss": np.ascontiguousarray(ssj.reshape(CK, 128).T),
            "mgi": _wrap16(mg),
            "e2i": _wrap16(e2),
            "opi": _wrap16(opf),
            "idn": ident,
        })
        own_b.append(bsel)

    return in_maps, own_b, NOCH


def _finish(results, own_b, NOCH):
    sum_d = 0.0
    sum_v = 0.0
    sum_v2 = 0.0
    d_own_all = np.empty(B, dtype=np.float64)
    n_total = 0
    for j in range(N_CORES):
        po = results[j]["po"].astype(np.float64)
        oo = results[j]["oo"].astype(np.float64)
        sum_d += po[:, 0].sum()
        sum_v += po[:, 1].sum()
        sum_v2 += po[:, 2].sum()
        bsel = own_b[j]
        vals = oo.T.ravel()[:len(bsel)]          # chunk-major: i = c*128 + p
        gp = vals / GSCALE
        d_own_all[bsel] = np.sqrt(np.maximum(0.0, 2.0 - 2.0 * gp))
        n_total += len(bsel)
    assert n_total == B

    intra = d_own_all.mean()
    viol_all = 2.0 * B * C - sum_d
    viol_own = np.maximum(0.0, 2.0 - d_own_all).sum()
    inter = (viol_all - viol_own) / (B * (C - 1))
    mean_v = sum_v / B
    var_v = max((sum_v2 - B * mean_v * mean_v) / (B - 1), 0.0)
    vdl = -np.sqrt(var_v)
    cr = -mean_v
    total = 1.0 * intra + 2.0 * inter + 0.5 * vdl + 0.1 * cr
    return (np.float32(total), np.float32(intra), np.float32(inter),
            np.float32(vdl), np.float32(cr), np.float32(intra))


USE_F32R = True


def kernel(geometric_stream, simplex_volumes, prototypes, labels):
    in_maps, own_b, NOCH = _prep(geometric_stream, simplex_volumes,
                                 prototypes, labels)
    nc = _build(NOCH, USE_F32R)
    res = run_bass_kernel_spmd(nc, in_maps, core_ids=list(range(N_CORES)))
    return _finish(res.results, own_b, NOCH)



# revision 42
# speedup vs baseline: 1.3341x; 1.3341x over previous
"""Trainium2 Bass kernel for nn_PrototypicalGeometricLoss.

Strategy (8 NeuronCores, single NEFF launch):
  - The host re-partitions the 4096 batch rows so that every class's members
    land on exactly one core (8 bins of exactly 512 rows, classes kept
    whole, rows sorted by class slot).  Segment sums for the EMA update are
    then core-local: no cross-core reduction of sums is needed.
  - Each core streams its 512 rows of geometric_stream (16.8 MB, the
    memory-bound part), mean-pools via a DVE/GPSIMD add-tree, L2-normalizes,
    and transposes to gT tiles.  Because rows arrive class-sorted, the
    prefix-scan segment-sum trick needs no member gather.
  - Distances are batch-sharded (each core: its 512 rows x all classes).
    For the ~2/3 of classes with count==0 the prototype is unchanged, so
    those distance matmuls (bf16) + ACT sqrt-accumulate run DURING the
    stream, hidden under the DMA.
  - The EMA update runs on each core for its own <=512 local classes in the
    [D, slot] orientation (renorm via gpsimd.partition_all_reduce).  The
    updated prototypes are shared via direct peer-to-peer SBUF DMA
    (remote_dma_broadcast, XOR-relative, one slot per peer so the seven
    64 KB fp8 transfers ride disjoint DMA-engine pairs) instead of a
    collective -- no ncfw latency floor.  A short tail computes distances
    for the updated classes after a semaphore wait on the peer writes.
  - Own-class dot products use the local updated prototypes (gather +
    elementwise product + partition_all_reduce); raw dots return to the
    host which finishes in f64.
  - Zero-padded prototype columns contribute sqrt(2) per pair; the device
    returns its own ACT-sqrt(2) so the host can subtract pads exactly.
"""

import functools
import sys

sys.path.insert(0, "/opt/trn_rl_repo")

import numpy as np

import concourse.bass as bass  # noqa: F401
import concourse.bass_interp as bass_interp
import concourse.bacc as bacc
import concourse.bass_isa as bass_isa
import concourse.mybir as mybir
from concourse import tile
from concourse.bass_utils import run_bass_kernel_spmd

N_CORES = 8
B, S, D, C = 4096, 64, 128, 10000
BSH = B // N_CORES           # 512 batch rows per core
LTB = BSH // 128             # 4 local b-tiles
SLOTS = 512                  # class slots per core (exactly 512 members)
EGL = 528                    # endpoint-gather length (513 used, 16-padded)
MOM = 0.9
GSCALE = 1.0 - 1e-6
EPS = 1e-12

f32 = mybir.dt.float32
bf16 = mybir.dt.bfloat16
fp8 = mybir.dt.float8e4
i16 = mybir.dt.int16
AF = mybir.ActivationFunctionType
ALU = mybir.AluOpType
AX = mybir.AxisListType
ROP = bass_isa.ReduceOp


def _wrap16(flat):
    """Lay a flat index list out in the GPSIMD wrapped-by-16 format."""
    n = flat.shape[0]
    assert n % 16 == 0
    w = flat.reshape(n // 16, 16).T.astype(np.int16)   # [16, n//16]
    return np.tile(w, (8, 1))                          # [128, n//16]


def _chunk_groups(total, group=1536):
    """Split `total` cols into PSUM-sized groups, each a list of <=1024-col
    matmul spans."""
    out = []
    c0 = 0
    while c0 < total:
        cn = min(group, total - c0)
        spans = []
        s0 = 0
        while s0 < cn:
            sn = min(512, cn - s0)
            spans.append((c0 + s0, sn))
            s0 += sn
        out.append((c0, cn, spans))
        c0 += cn
    return out


@functools.lru_cache(maxsize=8)
def _build(NNP, upto=99, unroll=1):
    """Build + compile the SPMD program.

    NNP = padded non-updated class count (multiple of 128).
    upto: 1=stream 2=+ema 3=+collective 4=+own-dots 5=+upd-dist (99=all).
    """
    nc = bacc.Bacc("TRN2", target_bir_lowering=False, debug=False,
                   detect_race_conditions=False, num_devices=N_CORES)

    rsem = nc.alloc_semaphore("rsem")
    lsem = nc.alloc_semaphore("lsem")

    return _build_body(nc, rsem, lsem, NNP, upto, unroll)


def _build_body(nc, rsem, lsem, NNP, upto, unroll):
    gs = nc.dram_tensor("gs", [BSH, S * D], f32, kind="ExternalInput")
    prn = nc.dram_tensor("prn", [128, NNP], fp8, kind="ExternalInput")
    prl = nc.dram_tensor("prl", [128, SLOTS], f32, kind="ExternalInput")
    cs = nc.dram_tensor("cs", [128, LTB], f32, kind="ExternalInput")
    sv = nc.dram_tensor("sv", [128, LTB], f32, kind="ExternalInput")
    epi = nc.dram_tensor("epi", [128, EGL // 16], i16, kind="ExternalInput")
    opi = nc.dram_tensor("opi", [128, SLOTS // 16], i16, kind="ExternalInput")
    idn = nc.dram_tensor("idn", [128, 128], f32, kind="ExternalInput")
    po = nc.dram_tensor("po", [128, 4], f32, kind="ExternalOutput")
    oo = nc.dram_tensor("oo", [1, SLOTS], f32, kind="ExternalOutput")

    gn_nonupd = _chunk_groups(NNP)
    gn_upd = _chunk_groups(N_CORES * SLOTS)
    n_acc = LTB * (len(gn_nonupd) + len(gn_upd))
    wait_fixups = []

    with tile.TileContext(nc) as tc:
        with (
            tc.tile_pool(name="dram", bufs=1, space="DRAM") as dram,
            tc.tile_pool(name="consts", bufs=1) as consts,
            tc.tile_pool(name="gbig", bufs=1) as gbig,
            tc.tile_pool(name="slab", bufs=3) as slabp,
            tc.tile_pool(name="norm", bufs=2) as normp,
            tc.tile_pool(name="ps_tr", bufs=2, space="PSUM") as ps_tr,
            tc.tile_pool(name="ps_dist", bufs=2, space="PSUM") as ps_dist,
            tc.tile_pool(name="dscr", bufs=2) as dscrp,
            tc.tile_pool(name="outs", bufs=1) as outsp,
        ):
            for it in range(unroll):
                if it > 0:
                    tc.strict_bb_all_engine_barrier()

                # DMA issue order controls transfer order on the shared DMA
                # resource: slab0 first, tiny consts, slab1, then the big
                # prototype matrix, then the remaining slabs; tail-only
                # consts last.
                def issue_slab(t):
                    sl = slabp.tile([128, S * D], f32, name="slab",
                                    tag="slab")
                    for qq in range(4):
                        nc.sync.dma_start(
                            sl[:, qq * 2048:(qq + 1) * 2048],
                            gs[t * 128:(t + 1) * 128,
                               qq * 2048:(qq + 1) * 2048])
                    return sl

                slab0 = issue_slab(0)

                ident = consts.tile([128, 128], f32, name="ident")
                nc.sync.dma_start(ident[:, :], idn[:, :])
                cs_sb = consts.tile([128, LTB], f32, name="cs_sb")
                nc.sync.dma_start(cs_sb[:, :], cs[:, :])
                sv_sb = consts.tile([128, LTB], f32, name="sv_sb")
                nc.sync.dma_start(sv_sb[:, :], sv[:, :])
                bias2 = consts.tile([128, 1], f32, name="bias2")
                nc.vector.memset(bias2[:, :], 2.0)
                epsb = consts.tile([128, 1], f32, name="epsb")
                nc.vector.memset(epsb[:, :], EPS)
                # slab1 ahead of the big prototype matrix so the stream
                # stays back-to-back; tail-only consts are issued at t==3.
                slab1 = issue_slab(1)
                prT = consts.tile([128, NNP], fp8, name="prT")
                nc.sync.dma_start(prT[:, :], prn[:, :])
                epi_sb = consts.tile([128, EGL // 16], i16, name="epi_sb")
                opi_sb = consts.tile([128, SLOTS // 16], i16, name="opi_sb")
                pr09 = consts.tile([128, SLOTS], f32, name="pr09")

                gwT = gbig.tile([128, EGL], f32, name="gwT")
                nc.vector.memset(gwT[:, :], 0.0)
                pTu = gbig.tile([128, N_CORES * SLOTS], fp8, name="pTu")
                ploc = gbig.tile([128, SLOTS], fp8, name="ploc")
                if upto >= 3:
                    # 6 of the 7 peer-broadcast descriptor preps issued
                    # early: their Q7 desc-gen runs under the stream.  The
                    # 7th (issued after the EMA write) carries the ploc
                    # dependency onto the trigger.
                    for r in range(1, N_CORES - 1):
                        rd = [None] * 8
                        rd[r] = (0, r)
                        nc.gpsimd.remote_dma_broadcast(
                            pTu[:, r * SLOTS:(r + 1) * SLOTS], ploc[:, :],
                            rsem, lsem, rdests=rd)
                gnT = gbig.tile([128, SLOTS], bf16, name="gnT")
                gnT8 = gbig.tile([128, SLOTS], fp8, name="gnT8")
                acc = outsp.tile([128, n_acc], f32, name="acc")
                out_sb = outsp.tile([128, 4], f32, name="out_sb")

                # ---- Phase A: stream + pool + normalize + transpose +
                #      non-updated-class distances (all under the DMA stream)
                for t in range(LTB if upto >= 1 else 0):
                    if t == 0:
                        slab = slab0
                    elif t == 1:
                        slab = slab1
                    else:
                        slab = issue_slab(t)
                    if t == 3:
                        nc.sync.dma_start(pr09[:, :], prl[:, :])
                        nc.vector.tensor_scalar_mul(pr09[:, :], pr09[:, :],
                                                    MOM)
                        nc.sync.dma_start(epi_sb[:, :], epi[:, :])
                        nc.sync.dma_start(opi_sb[:, :], opi[:, :])
                    pooled = normp.tile([128, 128], f32, name="pooled",
                                        tag="pooled")
                    ptmp = normp.tile([128, 128], f32, name="ptmp",
                                      tag="ptmp")
                    for qq in range(4):
                        qs = slab[:, qq * 2048:(qq + 1) * 2048]
                        eng = nc.gpsimd if qq in (0, 2) else nc.vector
                        for w in (1024, 512, 256, 128):
                            eng.tensor_tensor(qs[:, 0:w], qs[:, 0:w],
                                              qs[:, w:2 * w], ALU.add)
                    nc.vector.tensor_add(pooled[:, :], slab[:, 0:128],
                                         slab[:, 2048:2176])
                    nc.vector.tensor_add(ptmp[:, :], slab[:, 4096:4224],
                                         slab[:, 6144:6272])
                    nc.vector.tensor_add(pooled[:, :], pooled[:, :],
                                         ptmp[:, :])
                    scr = normp.tile([128, 128], f32, name="scr", tag="scr")
                    ssq = normp.tile([128, 1], f32, name="ssq", tag="ssq")
                    nc.scalar.activation(scr[:, :], pooled[:, :], AF.Square,
                                         accum_out=ssq[:, :])
                    nrm = normp.tile([128, 1], f32, name="nrm", tag="nrm")
                    nc.scalar.activation(nrm[:, :], ssq[:, :], AF.Sqrt)
                    nc.vector.tensor_scalar_max(nrm[:, :], nrm[:, :], EPS)
                    rcp = normp.tile([128, 1], f32, name="rcp", tag="rcp")
                    nc.vector.reciprocal(rcp[:, :], nrm[:, :])
                    gn = normp.tile([128, 128], f32, name="gn", tag="gn")
                    nc.vector.tensor_scalar(gn[:, :], pooled[:, :], rcp[:, :],
                                            GSCALE, ALU.mult, ALU.mult)
                    gw = normp.tile([128, 128], f32, name="gw", tag="gw")
                    nc.vector.tensor_scalar_mul(gw[:, :], gn[:, :],
                                                cs_sb[:, t:t + 1])
                    pst = ps_tr.tile([128, 128], f32, name="pst", tag="pst")
                    nc.tensor.transpose(pst[:, :], gn[:, :], ident[:, :])
                    nc.vector.tensor_scalar_mul(
                        gnT[:, t * 128:(t + 1) * 128], pst[:, :], 1.0)
                    nc.vector.tensor_scalar_mul(
                        gnT8[:, t * 128:(t + 1) * 128], pst[:, :], 1.0)
                    psw = ps_tr.tile([128, 128], f32, name="psw", tag="pst")
                    nc.tensor.transpose(psw[:, :], gw[:, :], ident[:, :])
                    nc.vector.tensor_scalar_mul(
                        gwT[:, 1 + t * 128:1 + (t + 1) * 128], psw[:, :], 1.0)

                    def emit_nonupd_dist(t):
                        lhs = gnT8[:, t * 128:(t + 1) * 128]
                        for gi, (g0, gcn, spans) in enumerate(gn_nonupd):
                            psd = ps_dist.tile([128, 1536], f32, name="psd",
                                               tag="psd")
                            for s0, sn in spans:
                                mm = nc.tensor.matmul(
                                    psd[:, s0 - g0:s0 - g0 + sn],
                                    lhs, prT[:, s0:s0 + sn],
                                    start=True, stop=True)
                            dsc = dscrp.tile([128, 1536], bf16, name="dsc",
                                             tag="dsc")
                            col = t * len(gn_nonupd) + gi
                            nc.scalar.activation(dsc[:, 0:gcn], psd[:, 0:gcn],
                                                 AF.Sqrt, bias=bias2[:, :],
                                                 scale=-2.0,
                                                 accum_out=acc[:, col:col + 1])
                    if t < 3:
                        emit_nonupd_dist(t)

                # ---- simplex volume partials + act-sqrt(2) reference value
                nc.vector.tensor_reduce(out_sb[:, 1:2], sv_sb[:, :], AX.X,
                                        ALU.add)
                junk1 = outsp.tile([128, LTB], f32, name="junk1")
                nc.vector.scalar_tensor_tensor(junk1[:, :], sv_sb[:, :], 1.0,
                                               sv_sb[:, :], ALU.mult,
                                               ALU.mult,
                                               accum_out=out_sb[:, 2:3])
                nc.scalar.activation(out_sb[:, 3:4], bias2[:, :], AF.Sqrt)

                # ---- Phase B: local segment sums + EMA + renorm
                plocf = gbig.tile([128, SLOTS], f32, name="plocf")
                if upto >= 2:
                    cum = gbig.tile([128, EGL], f32, name="cum")
                    nc.vector.tensor_tensor_scan(cum[:, :], gwT[:, :],
                                                 gwT[:, :], 0.0, ALU.add,
                                                 ALU.bypass)
                    fx = gbig.tile([128, EGL], f32, name="fx")
                    nc.gpsimd.ap_gather(fx[:, :], cum[:, :], epi_sb[:, :],
                                        channels=128, num_elems=EGL, d=1,
                                        num_idxs=EGL)
                    q = gbig.tile([128, SLOTS], f32, name="q")
                    nc.vector.tensor_sub(q[:, :], fx[:, 1:SLOTS + 1],
                                         fx[:, 0:SLOTS])
                    nc.vector.tensor_add(q[:, :], q[:, :], pr09[:, :])
                    sq = gbig.tile([128, SLOTS], f32, name="sq")
                    nc.vector.tensor_tensor(sq[:, :], q[:, :], q[:, :],
                                            ALU.mult)
                    s2 = gbig.tile([128, SLOTS], f32, name="s2")
                    nc.gpsimd.partition_all_reduce(s2[:, :], sq[:, :],
                                                   channels=128,
                                                   reduce_op=ROP.add)
                    pn = gbig.tile([128, SLOTS], f32, name="pn")
                    nc.scalar.activation(pn[:, :], s2[:, :], AF.Sqrt,
                                         bias=epsb[:, :])
                    rq = gbig.tile([128, SLOTS], f32, name="rq")
                    nc.vector.reciprocal(rq[:, :], pn[:, :])
                    nc.vector.tensor_tensor(plocf[:, :], q[:, :], rq[:, :],
                                            ALU.mult)
                    nc.vector.tensor_tensor(ploc[:, :], q[:, :], rq[:, :],
                                            ALU.mult)
                else:
                    nc.vector.memset(ploc[:, :], 0.0)
                    nc.vector.memset(plocf[:, :], 0.0)

                # ---- Phase C: share updated prototypes via direct
                #      peer-to-peer SBUF DMA (XOR-relative broadcast, one
                #      slot per peer => disjoint DMA-engine pairs).  Section
                #      order differs per core but distance sums are
                #      order-independent.
                if upto >= 3:
                    rd = [None] * 8
                    rd[N_CORES - 1] = (0, N_CORES - 1)
                    nc.gpsimd.remote_dma_broadcast(
                        pTu[:, (N_CORES - 1) * SLOTS:N_CORES * SLOTS],
                        ploc[:, :], rsem, lsem, rdests=rd)
                    nc.gpsimd.trigger_dma(count=None)
                    nc.vector.tensor_scalar_mul(pTu[:, 0:SLOTS], ploc[:, :],
                                                1.0)
                else:
                    nc.vector.memset(pTu[:, :], 0.0)

                # ---- Phase D: own-class raw dots (local; overlaps the AG)
                dots = outsp.tile([128, SLOTS], f32, name="dots")
                if upto >= 4:
                    opg = gbig.tile([128, SLOTS], f32, name="opg")
                    nc.gpsimd.ap_gather(opg[:, :], plocf[:, :], opi_sb[:, :],
                                        channels=128, num_elems=SLOTS, d=1,
                                        num_idxs=SLOTS)
                    prod = gbig.tile([128, SLOTS], f32, name="prod")
                    nc.vector.tensor_tensor(prod[:, :], opg[:, :], gnT[:, :],
                                            ALU.mult)
                    nc.gpsimd.partition_all_reduce(dots[:, :], prod[:, :],
                                                   channels=128,
                                                   reduce_op=ROP.add)
                else:
                    nc.vector.memset(dots[:, :], 0.0)
                nc.sync.dma_start(oo[:, :], dots[0:1, :])

                # ---- tile-3 non-updated distances (issued after the
                #      EMA chain so its ACT ops don't block the EMA sqrt;
                #      they fill ACT while the peer DMA is in flight)
                if upto >= 1:
                    emit_nonupd_dist(3)

                # ---- Phase E: updated-class distances, gated on the
                # peer transfers (2 sem increments per transfer x 7 peers
                # per iteration) via waits attached to the data-anchored
                # matmuls.
                base = LTB * len(gn_nonupd)
                for t in range(LTB if upto >= 5 else 0):
                    lhs = gnT8[:, t * 128:(t + 1) * 128]
                    for gi, (g0, gcn, spans) in enumerate(gn_upd):
                        psd = ps_dist.tile([128, 1536], f32, name="psd2",
                                           tag="psd")
                        for s0, sn in spans:
                            mm = nc.tensor.matmul(
                                psd[:, s0 - g0:s0 - g0 + sn],
                                lhs, pTu[:, s0:s0 + sn],
                                start=True, stop=True)
                            if upto >= 3:
                                # emitted with value 0 so the no_exec
                                # scheduling sim (which never fires RDMA
                                # transfers) stays satisfiable; rewritten
                                # below to the real receive-side target of
                                # 2 x 7 peer transfers per iteration
                                mm._wait_ge(rsem, 0)
                                wait_fixups.append((mm.ins, 14 * (it + 1)))
                        dsc = dscrp.tile([128, 1536], bf16, name="dsc2",
                                         tag="dsc")
                        col = base + t * len(gn_upd) + gi
                        nc.scalar.activation(dsc[:, 0:gcn], psd[:, 0:gcn],
                                             AF.Sqrt, bias=bias2[:, :],
                                             scale=-2.0,
                                             accum_out=acc[:, col:col + 1])
                if upto < 5 or upto < 1:
                    nc.vector.memset(acc[:, :], 0.0)

                nc.vector.tensor_reduce(out_sb[:, 0:1], acc[:, :], AX.X,
                                        ALU.add)
                nc.sync.dma_start(po[:, :], out_sb[:, :])

    for ins, val in wait_fixups:
        hits = 0
        for w in ins.sync_info.on_wait:
            if w.id == rsem.num:
                w.wait_value = val
                hits += 1
        assert hits == 1, (ins.name, hits)

    nc.compile()
    return nc


def _partition_classes(counts):
    """Assign each nonzero class to one of 8 bins of exactly BSH members."""
    upd = np.nonzero(counts)[0]
    sizes = counts[upd].astype(np.int64)
    order = np.argsort(-sizes, kind="stable")
    rng = np.random.default_rng(0)
    for attempt in range(200):
        rem = np.full(N_CORES, BSH, dtype=np.int64)
        bins = [[] for _ in range(N_CORES)]
        ok = True
        for i in order:
            j = int(np.argmax(rem))
            if rem[j] < sizes[i]:
                ok = False
                break
            bins[j].append(upd[i])
            rem[j] -= sizes[i]
        if ok and rem.sum() == 0 and (rem == 0).all():
            return [np.sort(np.array(b, dtype=np.int64)) for b in bins]
        order = rng.permutation(len(sizes))
        big = np.argsort(-sizes, kind="stable")[: len(sizes) // 4]
        order = np.concatenate([big, np.setdiff1d(order, big,
                                                  assume_unique=False)])
    raise RuntimeError("class partition failed")


def _prep(geometric_stream, simplex_volumes, prototypes, labels):
    gs = np.asarray(geometric_stream, dtype=np.float32)
    svol = np.asarray(simplex_volumes, dtype=np.float32).ravel()
    pr = np.asarray(prototypes, dtype=np.float32)
    lab = np.asarray(labels).astype(np.int64).ravel()
    assert gs.shape == (B, S, D) and pr.shape == (C, D) and lab.shape == (B,)

    counts = np.bincount(lab, minlength=C)
    bins = _partition_classes(counts)
    upd_all = np.nonzero(counts)[0]
    NU = len(upd_all)
    nonupd = np.setdiff1d(np.arange(C), upd_all, assume_unique=True)
    NN = len(nonupd)
    NNP = -(-NN // 128) * 128

    # Global non-updated prototype matrix, transposed, bf16, zero-padded.
    prn = np.zeros((128, NNP), dtype=np.float32)
    prn[:, :NN] = pr[nonupd].T
    prn_bf = to_fp8(prn)

    # order sorted by class; members of a class sorted by b
    by_class = {c: np.nonzero(lab == c)[0] for c in upd_all}

    in_maps = []
    member_b = []        # per core: batch idx per member position
    ident = np.eye(128, dtype=np.float32)
    for j in range(N_CORES):
        cls = bins[j]                     # sorted class ids for this core
        mlists = [by_class[c] for c in cls]
        msizes = np.array([len(m) for m in mlists], dtype=np.int64)
        bsel = (np.concatenate(mlists) if mlists else
                np.empty(0, dtype=np.int64))
        assert len(bsel) == BSH
        member_b.append(bsel)

        # endpoint gather: cum-index boundary after each slot
        mc = np.zeros(SLOTS + 1, dtype=np.int64)
        mc[1:len(cls) + 1] = np.cumsum(msizes)
        mc[len(cls) + 1:] = mc[len(cls)]
        e2 = np.full(EGL, mc[-1], dtype=np.int64)
        e2[:SLOTS + 1] = mc

        # own-prototype gather: class slot per member position
        slot_of = np.repeat(np.arange(len(cls), dtype=np.int64), msizes)
        opf = np.zeros(SLOTS, dtype=np.int64)
        opf[:len(slot_of)] = slot_of

        # local prototypes [D, slot], f32, zero-padded
        prj = np.zeros((128, SLOTS), dtype=np.float32)
        prj[:, :len(cls)] = pr[cls].T

        # per-member EMA weight (1-MOM)/count, tile-column layout
        cw = ((1.0 - MOM) / counts[lab[bsel]]).astype(np.float32)

        in_maps.append({
            "gs": gs[bsel].reshape(BSH, S * D),
            "prn": prn_bf,
            "prl": np.ascontiguousarray(prj),
            "cs": np.ascontiguousarray(cw.reshape(LTB, 128).T),
            "sv": np.ascontiguousarray(svol[bsel].reshape(LTB, 128).T),
            "epi": _wrap16(e2),
            "opi": _wrap16(opf),
            "idn": ident,
        })

    meta = {"NU": NU, "NN": NN, "NNP": NNP}
    return in_maps, member_b, meta


def to_fp8(a):
    import ml_dtypes
    return a.astype(ml_dtypes.float8_e4m3)


def _finish(results, member_b, meta):
    NU, NN, NNP = meta["NU"], meta["NN"], meta["NNP"]
    n_pad = (NNP - NN) + (N_CORES * SLOTS - NU)

    sum_d = 0.0
    sum_v = 0.0
    sum_v2 = 0.0
    d_own_all = np.empty(B, dtype=np.float64)
    n_total = 0
    for j in range(N_CORES):
        po = results[j]["po"].astype(np.float64)
        oo = results[j]["oo"].astype(np.float64)
        sqrt2 = po[0, 3]
        sum_d += po[:, 0].sum() - n_pad * BSH * sqrt2
        sum_v += po[:, 1].sum()
        sum_v2 += po[:, 2].sum()
        bsel = member_b[j]
        vals = oo.ravel()[:len(bsel)]        # member order
        gp = vals / GSCALE
        d_own_all[bsel] = np.sqrt(np.maximum(0.0, 2.0 - 2.0 * gp))
        n_total += len(bsel)
    assert n_total == B

    intra = d_own_all.mean()
    viol_all = 2.0 * B * C - sum_d
    viol_own = np.maximum(0.0, 2.0 - d_own_all).sum()
    inter = (viol_all - viol_own) / (B * (C - 1))
    mean_v = sum_v / B
    var_v = max((sum_v2 - B * mean_v * mean_v) / (B - 1), 0.0)
    vdl = -np.sqrt(var_v)
    cr = -mean_v
    total = 1.0 * intra + 2.0 * inter + 0.5 * vdl + 0.1 * cr
    return (np.float32(total), np.float32(intra), np.float32(inter),
            np.float32(vdl), np.float32(cr), np.float32(intra))


def kernel(geometric_stream, simplex_volumes, prototypes, labels):
    in_maps, member_b, meta = _prep(geometric_stream, simplex_volumes,
                                    prototypes, labels)
    nc = _build(meta["NNP"])
    res = run_bass_kernel_spmd(nc, in_maps, core_ids=list(range(N_CORES)))
    return _finish(res.results, member_b, meta)
